# revision 17
# baseline (speedup 1.0000x reference)
"""GATv2 message passing (nn_KG_GNN_84430467105347) on 8 Trainium2 NeuronCores.

Strategy (dst-sharded, host-pregathered edge features, no collectives):
  - Host: append self-loops, sort edges by destination, shard by dst range
    (12544 = 98*128 nodes per core). Each core owns the full softmax +
    aggregation for its dst range; outputs are concatenated on the host.
  - Host pre-gathers BOTH endpoints' raw features per edge (x[src], x[dst]),
    transposed to [128 feat, edge] fp16, plus the one-hot destination-slot
    masks; the device computes m = x_s@W_l + x_d@W_r per 128-edge chunk with
    two PE matmuls straight into PSUM. No gathers, no x_l/x_r tables.
  - Softmax-sum identity: since softmax weights sum to 1 per (dst, head),
    aggregating m = x_l + x_r[dst] instead of x_l gives
    out[d] = sum_e alpha_e*m_e - x_r[d]; the x_r correction is applied once
    per 128-node tile (one matmul from the resident transposed local x).
  - Scores via ONE fused custom-DVE op: prefix-scan of
    att * leaky_relu(m) over the group's 1024 features, then a boundary
    difference extracts the 32-feature head sums. exp on the scalar engine.
  - Numerator via a second fused custom-DVE op: m * exp (page-broadcast).
  - Scatter-softmax + aggregation per dst tile via one PE matmul per chunk:
    out_ps[dst, 0:128|128:132] += mask_b^T @ [exp*m | exp], PSUM-accumulated
    across the tile's chunks, then normalized, corrected, written out.
  - The static chunk schedule (chunks per tile = max over cores) is computed
    from the actual edge data at kernel() time; one SPMD program serves all
    8 cores. Padded slots have all-zero mask rows.
"""
import sys
sys.path.insert(0, '/opt/trn_rl_repo')
import numpy as np

N_NODES = 100000
IN_DIM = 128
H, C = 4, 32
F = 128           # = H*C = IN_DIM
P = 128
NEG_SLOPE = 0.2
N_CORES = 8
NT = 98           # node tiles per core
NPC = NT * P      # 12544 nodes per core
NPAD = N_CORES * NPC
GB = 8            # chunks per group (PSUM: m_ps = GB*512B/partition)

LAST_NC = None
_OPS_REGISTERED = {}


def _register_dve_ops():
    if _OPS_REGISTERED:
        return _OPS_REGISTERED
    from concourse import dve_ops
    from concourse.dve_spec import Spec, Src0, Src1, C0, maxx, scan, lower, AluOp
    from concourse.dve_uop import DveOpSpec

    def reg(name, spec, subdim):
        if name in dve_ops._SUB_OPCODE_FOR_NAME:
            for op in dve_ops.OPS:
                if op.name == name:
                    return op
        op = dve_ops.DveOp(name, spec, subdim=subdim, uops_sha={})
        dve_ops.OPS.append(op)
        dve_ops.CUSTOM_DVE_SPECS[name] = spec
        dve_ops._SUB_OPCODE_FOR_NAME[name] = (dve_ops._CUSTOM_DVE_ROW_BASE
                                              + len(dve_ops.OPS) - 1)
        from concourse.dve_ops import get_dve_sub_opcode, has_src1
        for ver in ("v3", "v4"):
            s = DveOpSpec(name=name, opcode=get_dve_sub_opcode(name),
                          uops=lower(spec, ver=ver), rd1_en=has_src1(spec))
            op.uops_sha[ver] = s.sha(ver)
        return op

    _OPS_REGISTERED["scan"] = reg(
        "GNN_SCORE_SCAN",
        Spec(body=scan(AluOp.ADD, maxx(Src0, Src0 * C0) * Src1),
             reference=lambda in0, in1, s0: np.cumsum(
                 np.maximum(in0, in0 * s0) * in1, axis=-1)),
        subdim=False)
    _OPS_REGISTERED["pmul"] = reg(
        "GNN_PAGED_MULT",
        Spec(body=Src0 * Src1, reference=lambda in0, in1: in0 * in1),
        subdim=False)
    return _OPS_REGISTERED


def _host_prep(src, dst):
    """Sort by dst, shard by dst range, build per-core static chunk layout."""
    N = N_NODES
    s = np.concatenate([np.asarray(src, dtype=np.int64),
                        np.arange(N, dtype=np.int64)])
    d = np.concatenate([np.asarray(dst, dtype=np.int64),
                        np.arange(N, dtype=np.int64)])
    order = np.argsort(d, kind='stable')
    s = s[order].astype(np.int32)
    d = d[order].astype(np.int32)
    core = d // NPC
    tile_of = (d % NPC) // P
    slot_of = d % P
    counts = np.zeros((N_CORES, NT), dtype=np.int64)
    np.add.at(counts, (core, tile_of), 1)
    cpt = np.maximum(1, -(-counts.max(axis=0) // P))      # chunks per tile
    nchunk = int(cpt.sum())
    pad_groups = (-nchunk) % GB
    cpt[NT - 1] += pad_groups                              # pad to multiple of GB
    nchunk += pad_groups
    cbase = np.zeros(NT + 1, dtype=np.int64)
    np.cumsum(cpt, out=cbase[1:])

    src_flat = np.zeros((N_CORES, nchunk * P), dtype=np.int32)
    dst_flat = np.zeros((N_CORES, nchunk * P), dtype=np.int32)
    slot_flat = np.full((N_CORES, nchunk * P), 255, dtype=np.int32)
    core_starts = np.searchsorted(core, np.arange(N_CORES + 1))
    for k in range(N_CORES):
        lo, hi = core_starts[k], core_starts[k + 1]
        sk, dk, tk, slk = s[lo:hi], d[lo:hi], tile_of[lo:hi], slot_of[lo:hi]
        tile_starts = np.searchsorted(tk, np.arange(NT + 1))
        for t in range(NT):
            a, b = tile_starts[t], tile_starts[t + 1]
            n = b - a
            if n == 0:
                continue
            pos = cbase[t] * P + np.arange(n)              # linear slot
            idx = (pos // P) * P + (pos % P)
            src_flat[k, idx] = sk[a:b]
            dst_flat[k, idx] = dk[a:b]
            slot_flat[k, idx] = slk[a:b]
    return src_flat, dst_flat, slot_flat, cpt, cbase, nchunk


def _build_program(nchunk, cpt, cbase, hb_lr):
    import concourse.mybir as mybir
    import concourse.tile as tile
    from concourse import bacc

    ops = _register_dve_ops()
    f16 = mybir.dt.float16
    f32 = mybir.dt.float32

    nc = bacc.Bacc(None, target_bir_lowering=False)
    # per group: [xsT | xdT | mask] interleaved, one DMA load
    blob_in = nc.dram_tensor("blob", [P, nchunk * P * 3], f16,
                             kind="ExternalInput")
    xlocT_in = nc.dram_tensor("xlocT", [P, NPC], f16, kind="ExternalInput")
    wl_in = nc.dram_tensor("wl", [P, F], f16, kind="ExternalInput")
    wr_in = nc.dram_tensor("wr", [P, F], f16, kind="ExternalInput")
    blr_in = nc.dram_tensor("blr_row", [1, F], f16, kind="ExternalInput")
    brmb_in = nc.dram_tensor("brmb_row", [1, F], f32, kind="ExternalInput")
    att_in = nc.dram_tensor("att_grp", [P, GB * F], f16, kind="ExternalInput")
    ones_in = nc.dram_tensor("ones_row", [1, P], f16, kind="ExternalInput")
    out_dram = nc.dram_tensor("out", [NPC, F], f32, kind="ExternalOutput")

    t_of = np.zeros(nchunk, dtype=np.int64)
    for t in range(NT):
        t_of[cbase[t]:cbase[t + 1]] = t

    with tile.TileContext(nc) as tc:
        with tc.tile_pool(name="persist", bufs=1) as pp:
            att_grp = pp.tile([P, GB * F], f16)
            nc.sync.dma_start(att_grp[:], att_in[:])
            wl16 = pp.tile([P, F], f16)
            nc.sync.dma_start(wl16[:], wl_in[:])
            wr16 = pp.tile([P, F], f16)
            nc.sync.dma_start(wr16[:], wr_in[:])
            blr = pp.tile([1, F], f16)
            nc.sync.dma_start(blr[:], blr_in[:])
            brmb = pp.tile([1, F], f32)
            nc.sync.dma_start(brmb[:], brmb_in[:])
            ones16 = pp.tile([1, P], f16)
            nc.sync.dma_start(ones16[:], ones_in[:])
            onesf = pp.tile([1, P], f32)
            nc.vector.tensor_copy(onesf[:], ones16[:])
            xlocT_sb = pp.tile([P, NPC], f16)
            nc.sync.dma_start(xlocT_sb[:], xlocT_in[:])

            with tc.tile_pool(name="eg_sb", bufs=3) as sb, \
                 tc.tile_pool(name="m_ps", bufs=2, space="PSUM") as eps, \
                 tc.tile_pool(name="o_ps", bufs=2, space="PSUM") as ops_ps, \
                 tc.tile_pool(name="x_ps", bufs=2, space="PSUM") as xps, \
                 tc.tile_pool(name="out_sb", bufs=3) as osb:
                ngroups = nchunk // GB
                out_ps = None
                for g in range(ngroups):
                    c0 = g * GB
                    blob = sb.tile([P, 3 * GB * P], f16, tag="blob")
                    nc.sync.dma_start(
                        blob[:], blob_in[:, c0 * P * 3:(c0 + GB) * P * 3])
                    xsT = blob[:, :GB * P]
                    xdT = blob[:, GB * P:2 * GB * P]
                    mask = blob[:, 2 * GB * P:].rearrange(
                        "p (b d) -> p b d", d=P)
                    m_ps = eps.tile([P, GB, F], f32, tag="m", space="PSUM")
                    for b in range(GB):
                        nc.tensor.matmul(out=m_ps[:, b, :],
                                         lhsT=xsT[:, b * P:(b + 1) * P],
                                         rhs=wl16[:], start=True, stop=False)
                        if hb_lr:
                            nc.tensor.matmul(out=m_ps[:, b, :], lhsT=ones16[:],
                                             rhs=blr[:], start=False,
                                             stop=False)
                        nc.tensor.matmul(out=m_ps[:, b, :],
                                         lhsT=xdT[:, b * P:(b + 1) * P],
                                         rhs=wr16[:], start=False, stop=True)
                    m16 = sb.tile([P, GB, F], f16, tag="m16")
                    nc.scalar.copy(m16[:], m_ps[:])
                    prefix = sb.tile([P, 32 + GB * F], f32, tag="prefix")
                    nc.vector.memset(prefix[:, 31:32], 0.0)
                    nc.vector._custom_dve(
                        ops["scan"], out=prefix[:, 32:],
                        in0=m16[:].rearrange("p b f -> p (b f)"),
                        in1=att_grp[:], s0=NEG_SLOPE)
                    esc = sb.tile([P, GB * H], f32, tag="esc")
                    pv = prefix[:].rearrange("p (a c) -> p a c", c=C)
                    nc.vector.tensor_tensor(
                        out=esc[:], in0=pv[:, 1:, C - 1], in1=pv[:, :GB * H, C - 1],
                        op=mybir.AluOpType.subtract)
                    # rhsw layout: per (chunk, head) page = [32 feats | exp]
                    rhsw = sb.tile([P, GB, H, C + 1], f16, tag="rhsw")
                    nc.scalar.activation(
                        out=rhsw[:].rearrange("p b h x -> p (b h) x")
                            [:, :, C:],
                        in_=esc[:, :, None],
                        func=mybir.ActivationFunctionType.Exp)
                    nc.vector._custom_dve(
                        ops["pmul"],
                        out=rhsw[:].rearrange("p b h x -> p (b h) x")
                            [:, :, :C],
                        in0=m16[:].rearrange("p b (h c) -> p (b h) c", h=H),
                        in1=rhsw[:].rearrange("p b h x -> p (b h) x")
                            [:, :, C:].to_broadcast([P, GB * H, C]))
                    for b in range(GB):
                        c = c0 + b
                        t = int(t_of[c])
                        if c == cbase[t]:
                            out_ps = ops_ps.tile([P, F + H], f32, tag="out",
                                                 space="PSUM")
                        nc.tensor.matmul(out=out_ps[:], lhsT=mask[:, b, :],
                                         rhs=rhsw[:, b, :, :].rearrange(
                                             "p h x -> p (h x)"),
                                         start=(c == cbase[t]),
                                         stop=(c == cbase[t + 1] - 1))
                        if c == cbase[t + 1] - 1:
                            # per-tile x_r correction: xrb = x_loc@W_r + b_r - bias
                            xrb_ps = xps.tile([P, F], f32, tag="xrb",
                                              space="PSUM")
                            nc.tensor.matmul(
                                out=xrb_ps[:],
                                lhsT=xlocT_sb[:, t * P:(t + 1) * P],
                                rhs=wr16[:], start=True, stop=False)
                            nc.tensor.matmul(out=xrb_ps[:], lhsT=onesf[:],
                                             rhs=brmb[:], start=False,
                                             stop=True)
                            opsv = out_ps[:].rearrange("p (h x) -> p h x",
                                                       h=H)
                            den = osb.tile([P, H], f32, tag="den")
                            nc.vector.tensor_scalar_max(den[:],
                                                        opsv[:, :, C:],
                                                        1e-30)
                            recip = osb.tile([P, H], f32, tag="recip")
                            nc.vector.reciprocal(recip[:], den[:])
                            fin = osb.tile([P, F], f32, tag="fin")
                            nc.vector.tensor_tensor(
                                out=fin[:].rearrange("p (h c) -> p h c", h=H),
                                in0=opsv[:, :, :C],
                                in1=recip[:, :, None].to_broadcast([P, H, C]),
                                op=mybir.AluOpType.mult)
                            fin2 = osb.tile([P, F], f32, tag="fin2")
                            nc.vector.tensor_tensor(
                                out=fin2[:], in0=fin[:], in1=xrb_ps[:],
                                op=mybir.AluOpType.subtract)
                            nc.sync.dma_start(out_dram[t * P:(t + 1) * P, :],
                                              fin2[:])
    nc.compile()
    return nc


def _make_in_maps(x, W_l, b_l, W_r, b_r, att, bias, src_flat, dst_flat,
                  slot_flat, nchunk):
    x16 = np.zeros((NPAD, IN_DIM), dtype=np.float16)
    x16[:N_NODES] = x.astype(np.float16)
    wl = np.ascontiguousarray(W_l.astype(np.float16))
    wr = np.ascontiguousarray(W_r.astype(np.float16))
    blr = (b_l + b_r)[None, :].astype(np.float16)
    brmb = (b_r - bias)[None, :].astype(np.float32)
    att_grp = np.tile(att.reshape(1, F), (P, GB)).astype(np.float16)
    ones_row = np.ones((1, P), dtype=np.float16)
    iota = np.arange(P, dtype=np.int32)
    ngroups = nchunk // GB
    W = GB * P
    in_maps = []
    for k in range(N_CORES):
        xsT = x16[src_flat[k]].T                          # [P, nchunk*P]
        xdT = x16[dst_flat[k]].T
        # one-hot mask rows: [P, nchunk*P]; mask[p, c*P + d] = slot(p,c)==d
        sl = slot_flat[k].reshape(nchunk, P)              # [c, p]
        maskH = (sl.T[:, :, None] == iota[None, None, :]).astype(
            np.float16).reshape(P, nchunk * P)
        # interleave per group: [xsT_g | xdT_g | mask_g]
        blob = np.empty((P, ngroups, 3, W), dtype=np.float16)
        blob[:, :, 0, :] = xsT.reshape(P, ngroups, W)
        blob[:, :, 1, :] = xdT.reshape(P, ngroups, W)
        blob[:, :, 2, :] = maskH.reshape(P, ngroups, W)
        xlocT = np.ascontiguousarray(x16[k * NPC:(k + 1) * NPC].T)
        in_maps.append({
            "blob": blob.reshape(P, nchunk * P * 3), "xlocT": xlocT,
            "wl": wl, "wr": wr, "blr_row": blr, "brmb_row": brmb,
            "att_grp": att_grp, "ones_row": ones_row,
        })
    return in_maps


def kernel(x, W_l, b_l, W_r, b_r, att, bias, src, dst):
    x = np.asarray(x, dtype=np.float32)
    W_l = np.asarray(W_l, dtype=np.float32)
    W_r = np.asarray(W_r, dtype=np.float32)
    b_l = np.asarray(b_l, dtype=np.float32)
    b_r = np.asarray(b_r, dtype=np.float32)
    att = np.asarray(att, dtype=np.float32)
    bias = np.asarray(bias, dtype=np.float32)

    src_flat, dst_flat, slot_flat, cpt, cbase, nchunk = _host_prep(src, dst)
    hb_lr = bool(np.any(b_l != 0) or np.any(b_r != 0))
    nc = _build_program(nchunk, cpt, cbase, hb_lr)
    in_maps = _make_in_maps(x, W_l, b_l, W_r, b_r, att, bias,
                            src_flat, dst_flat, slot_flat, nchunk)

    global LAST_NC
    LAST_NC = nc
    from concourse import bass2jax
    results = bass2jax.run_bass_via_pjrt(nc, in_maps, n_cores=N_CORES)

    out = np.empty((N_NODES, F), dtype=np.float32)
    for k in range(N_CORES):
        lo = k * NPC
        hi = min(lo + NPC, N_NODES)
        out[lo:hi] = results[k]["out"][:hi - lo]
    return out


# revision 18
# speedup vs baseline: 1.5573x; 1.5573x over previous
"""GATv2 message passing (nn_KG_GNN_84430467105347) on 8 Trainium2 NeuronCores.

Strategy (dst-sharded, host-pregathered edge features, no collectives):
  - Host: append self-loops, sort edges by destination, shard by dst range
    (12544 = 98*128 nodes per core). Each core owns the full softmax +
    aggregation for its dst range; outputs are concatenated on the host.
  - Host pre-gathers BOTH endpoints' raw features per edge (x[src], x[dst]),
    transposed to [128 feat, edge] fp16, plus the one-hot destination-slot
    masks; the device computes m = x_s@W_l + x_d@W_r per 128-edge chunk with
    two PE matmuls straight into PSUM. No gathers, no x_l/x_r tables.
  - Softmax-sum identity: since softmax weights sum to 1 per (dst, head),
    aggregating m = x_l + x_r[dst] instead of x_l gives
    out[d] = sum_e alpha_e*m_e - x_r[d]; the x_r correction is applied once
    per 128-node tile (one matmul from the resident transposed local x).
  - Scores via ONE fused custom-DVE op: prefix-scan of
    att * leaky_relu(m) over the group's 1024 features, then a boundary
    difference extracts the 32-feature head sums. exp on the scalar engine.
  - Numerator via a second fused custom-DVE op: m * exp (page-broadcast).
  - Scatter-softmax + aggregation per dst tile via one PE matmul per chunk:
    out_ps[dst, 0:128|128:132] += mask_b^T @ [exp*m | exp], PSUM-accumulated
    across the tile's chunks, then normalized, corrected, written out.
  - The static chunk schedule (chunks per tile = max over cores) is computed
    from the actual edge data at kernel() time; one SPMD program serves all
    8 cores. Padded slots have all-zero mask rows.
"""
import sys
sys.path.insert(0, '/opt/trn_rl_repo')
import numpy as np

N_NODES = 100000
IN_DIM = 128
H, C = 4, 32
F = 128           # = H*C = IN_DIM
P = 128
NEG_SLOPE = 0.2
N_CORES = 8
NT = 98           # node tiles per core
NPC = NT * P      # 12544 nodes per core
NPAD = N_CORES * NPC
GB = 8            # chunks per group (PSUM: m_ps = GB*512B/partition)

LAST_NC = None
_OPS_REGISTERED = {}


def _register_dve_ops():
    if _OPS_REGISTERED:
        return _OPS_REGISTERED
    from concourse import dve_ops
    from concourse.dve_spec import Spec, Src0, Src1, C0, maxx, scan, lower, AluOp
    from concourse.dve_uop import DveOpSpec

    def reg(name, spec, subdim):
        if name in dve_ops._SUB_OPCODE_FOR_NAME:
            for op in dve_ops.OPS:
                if op.name == name:
                    return op
        op = dve_ops.DveOp(name, spec, subdim=subdim, uops_sha={})
        dve_ops.OPS.append(op)
        dve_ops.CUSTOM_DVE_SPECS[name] = spec
        dve_ops._SUB_OPCODE_FOR_NAME[name] = (dve_ops._CUSTOM_DVE_ROW_BASE
                                              + len(dve_ops.OPS) - 1)
        from concourse.dve_ops import get_dve_sub_opcode, has_src1
        for ver in ("v3", "v4"):
            s = DveOpSpec(name=name, opcode=get_dve_sub_opcode(name),
                          uops=lower(spec, ver=ver), rd1_en=has_src1(spec))
            op.uops_sha[ver] = s.sha(ver)
        return op

    _OPS_REGISTERED["scan"] = reg(
        "GNN_SCORE_SCAN",
        Spec(body=scan(AluOp.ADD, maxx(Src0, Src0 * C0) * Src1),
             reference=lambda in0, in1, s0: np.cumsum(
                 np.maximum(in0, in0 * s0) * in1, axis=-1)),
        subdim=False)
    _OPS_REGISTERED["pmul"] = reg(
        "GNN_PAGED_MULT",
        Spec(body=Src0 * Src1, reference=lambda in0, in1: in0 * in1),
        subdim=False)
    return _OPS_REGISTERED


def _host_prep(src, dst):
    """Sort by dst, shard by dst range, build per-core static chunk layout."""
    N = N_NODES
    s = np.concatenate([np.asarray(src, dtype=np.int64),
                        np.arange(N, dtype=np.int64)])
    d = np.concatenate([np.asarray(dst, dtype=np.int64),
                        np.arange(N, dtype=np.int64)])
    order = np.argsort(d, kind='stable')
    s = s[order].astype(np.int32)
    d = d[order].astype(np.int32)
    core = d // NPC
    tile_of = (d % NPC) // P
    slot_of = d % P
    counts = np.zeros((N_CORES, NT), dtype=np.int64)
    np.add.at(counts, (core, tile_of), 1)
    cpt = np.maximum(1, -(-counts.max(axis=0) // P))      # chunks per tile
    nchunk = int(cpt.sum())
    pad_groups = (-nchunk) % GB
    cpt[NT - 1] += pad_groups                              # pad to multiple of GB
    nchunk += pad_groups
    cbase = np.zeros(NT + 1, dtype=np.int64)
    np.cumsum(cpt, out=cbase[1:])

    src_flat = np.zeros((N_CORES, nchunk * P), dtype=np.int32)
    dst_flat = np.zeros((N_CORES, nchunk * P), dtype=np.int32)
    slot_flat = np.full((N_CORES, nchunk * P), 255, dtype=np.int32)
    core_starts = np.searchsorted(core, np.arange(N_CORES + 1))
    for k in range(N_CORES):
        lo, hi = core_starts[k], core_starts[k + 1]
        sk, dk, tk, slk = s[lo:hi], d[lo:hi], tile_of[lo:hi], slot_of[lo:hi]
        tile_starts = np.searchsorted(tk, np.arange(NT + 1))
        for t in range(NT):
            a, b = tile_starts[t], tile_starts[t + 1]
            n = b - a
            if n == 0:
                continue
            pos = cbase[t] * P + np.arange(n)              # linear slot
            idx = (pos // P) * P + (pos % P)
            src_flat[k, idx] = sk[a:b]
            dst_flat[k, idx] = dk[a:b]
            slot_flat[k, idx] = slk[a:b]
    return src_flat, dst_flat, slot_flat, cpt, cbase, nchunk


def _build_program(nchunk, cpt, cbase, hb_lr):
    import concourse.mybir as mybir
    import concourse.tile as tile
    from concourse import bacc

    ops = _register_dve_ops()
    f16 = mybir.dt.float16
    f32 = mybir.dt.float32

    nc = bacc.Bacc(None, target_bir_lowering=False)
    # per group: [xsT | xdT | mask] interleaved, one DMA load
    blob_in = nc.dram_tensor("blob", [P, nchunk * P * 3], f16,
                             kind="ExternalInput")
    xlocT_in = nc.dram_tensor("xlocT", [P, NPC], f16, kind="ExternalInput")
    wl_in = nc.dram_tensor("wl", [P, F], f16, kind="ExternalInput")
    wr_in = nc.dram_tensor("wr", [P, F], f16, kind="ExternalInput")
    blr_in = nc.dram_tensor("blr_row", [1, F], f16, kind="ExternalInput")
    brmb_in = nc.dram_tensor("brmb_row", [1, F], f32, kind="ExternalInput")
    att_in = nc.dram_tensor("att_grp", [P, GB * F], f16, kind="ExternalInput")
    ones_in = nc.dram_tensor("ones_row", [1, P], f16, kind="ExternalInput")
    out_dram = nc.dram_tensor("out", [NPC, F], f32, kind="ExternalOutput")

    t_of = np.zeros(nchunk, dtype=np.int64)
    for t in range(NT):
        t_of[cbase[t]:cbase[t + 1]] = t

    with tile.TileContext(nc) as tc:
        with tc.tile_pool(name="persist", bufs=1) as pp:
            att_grp = pp.tile([P, GB * F], f16)
            nc.sync.dma_start(att_grp[:], att_in[:])
            wl16 = pp.tile([P, F], f16)
            nc.sync.dma_start(wl16[:], wl_in[:])
            wr16 = pp.tile([P, F], f16)
            nc.sync.dma_start(wr16[:], wr_in[:])
            blr = pp.tile([1, F], f16)
            nc.sync.dma_start(blr[:], blr_in[:])
            brmb = pp.tile([1, F], f32)
            nc.sync.dma_start(brmb[:], brmb_in[:])
            ones16 = pp.tile([1, P], f16)
            nc.sync.dma_start(ones16[:], ones_in[:])
            onesf = pp.tile([1, P], f32)
            nc.vector.tensor_copy(onesf[:], ones16[:])
            xlocT_sb = pp.tile([P, NPC], f16)
            nc.sync.dma_start(xlocT_sb[:], xlocT_in[:])

            with tc.tile_pool(name="eg_sb", bufs=3) as sb, \
                 tc.tile_pool(name="m_ps", bufs=2, space="PSUM") as eps, \
                 tc.tile_pool(name="o_ps", bufs=2, space="PSUM") as ops_ps, \
                 tc.tile_pool(name="x_ps", bufs=2, space="PSUM") as xps, \
                 tc.tile_pool(name="out_sb", bufs=3) as osb:
                ngroups = nchunk // GB
                out_ps = None
                for g in range(ngroups):
                    c0 = g * GB
                    xsT = sb.tile([P, GB * P], f16, tag="xsT")
                    nc.sync.dma_start(
                        xsT[:], blob_in[:, c0 * P * 3:c0 * P * 3 + GB * P])
                    xdT = sb.tile([P, GB * P], f16, tag="xdT")
                    nc.sync.dma_start(
                        xdT[:], blob_in[:, c0 * P * 3 + GB * P:
                                        c0 * P * 3 + 2 * GB * P])
                    mask = sb.tile([P, GB, P], f16, tag="mask")
                    nc.sync.dma_start(
                        mask[:].rearrange("p b d -> p (b d)"),
                        blob_in[:, c0 * P * 3 + 2 * GB * P:(c0 + GB) * P * 3])
                    m_ps = eps.tile([P, GB, F], f32, tag="m", space="PSUM")
                    for b in range(GB):
                        nc.tensor.matmul(out=m_ps[:, b, :],
                                         lhsT=xsT[:, b * P:(b + 1) * P],
                                         rhs=wl16[:], start=True, stop=False)
                        if hb_lr:
                            nc.tensor.matmul(out=m_ps[:, b, :], lhsT=ones16[:],
                                             rhs=blr[:], start=False,
                                             stop=False)
                        nc.tensor.matmul(out=m_ps[:, b, :],
                                         lhsT=xdT[:, b * P:(b + 1) * P],
                                         rhs=wr16[:], start=False, stop=True)
                    m16 = sb.tile([P, GB, F], f16, tag="m16")
                    nc.scalar.copy(m16[:], m_ps[:])
                    prefix = sb.tile([P, 32 + GB * F], f32, tag="prefix")
                    nc.vector.memset(prefix[:, 31:32], 0.0)
                    nc.vector._custom_dve(
                        ops["scan"], out=prefix[:, 32:],
                        in0=m16[:].rearrange("p b f -> p (b f)"),
                        in1=att_grp[:], s0=NEG_SLOPE)
                    esc = sb.tile([P, GB * H], f32, tag="esc")
                    pv = prefix[:].rearrange("p (a c) -> p a c", c=C)
                    nc.vector.tensor_tensor(
                        out=esc[:], in0=pv[:, 1:, C - 1], in1=pv[:, :GB * H, C - 1],
                        op=mybir.AluOpType.subtract)
                    # rhsw layout: per (chunk, head) page = [32 feats | exp]
                    rhsw = sb.tile([P, GB, H, C + 1], f16, tag="rhsw")
                    nc.scalar.activation(
                        out=rhsw[:].rearrange("p b h x -> p (b h) x")
                            [:, :, C:],
                        in_=esc[:, :, None],
                        func=mybir.ActivationFunctionType.Exp)
                    nc.vector._custom_dve(
                        ops["pmul"],
                        out=rhsw[:].rearrange("p b h x -> p (b h) x")
                            [:, :, :C],
                        in0=m16[:].rearrange("p b (h c) -> p (b h) c", h=H),
                        in1=rhsw[:].rearrange("p b h x -> p (b h) x")
                            [:, :, C:].to_broadcast([P, GB * H, C]))
                    for b in range(GB):
                        c = c0 + b
                        t = int(t_of[c])
                        if c == cbase[t]:
                            out_ps = ops_ps.tile([P, F + H], f32, tag="out",
                                                 space="PSUM")
                        nc.tensor.matmul(out=out_ps[:], lhsT=mask[:, b, :],
                                         rhs=rhsw[:, b, :, :].rearrange(
                                             "p h x -> p (h x)"),
                                         start=(c == cbase[t]),
                                         stop=(c == cbase[t + 1] - 1))
                        if c == cbase[t + 1] - 1:
                            # per-tile x_r correction: xrb = x_loc@W_r + b_r - bias
                            xrb_ps = xps.tile([P, F], f32, tag="xrb",
                                              space="PSUM")
                            nc.tensor.matmul(
                                out=xrb_ps[:],
                                lhsT=xlocT_sb[:, t * P:(t + 1) * P],
                                rhs=wr16[:], start=True, stop=False)
                            nc.tensor.matmul(out=xrb_ps[:], lhsT=onesf[:],
                                             rhs=brmb[:], start=False,
                                             stop=True)
                            opsv = out_ps[:].rearrange("p (h x) -> p h x",
                                                       h=H)
                            den = osb.tile([P, H], f32, tag="den")
                            nc.vector.tensor_scalar_max(den[:],
                                                        opsv[:, :, C:],
                                                        1e-30)
                            recip = osb.tile([P, H], f32, tag="recip")
                            nc.vector.reciprocal(recip[:], den[:])
                            fin = osb.tile([P, F], f32, tag="fin")
                            nc.vector.tensor_tensor(
                                out=fin[:].rearrange("p (h c) -> p h c", h=H),
                                in0=opsv[:, :, :C],
                                in1=recip[:, :, None].to_broadcast([P, H, C]),
                                op=mybir.AluOpType.mult)
                            fin2 = osb.tile([P, F], f32, tag="fin2")
                            nc.vector.tensor_tensor(
                                out=fin2[:], in0=fin[:], in1=xrb_ps[:],
                                op=mybir.AluOpType.subtract)
                            nc.sync.dma_start(out_dram[t * P:(t + 1) * P, :],
                                              fin2[:])
    nc.compile()
    return nc


def _make_in_maps(x, W_l, b_l, W_r, b_r, att, bias, src_flat, dst_flat,
                  slot_flat, nchunk):
    x16 = np.zeros((NPAD, IN_DIM), dtype=np.float16)
    x16[:N_NODES] = x.astype(np.float16)
    wl = np.ascontiguousarray(W_l.astype(np.float16))
    wr = np.ascontiguousarray(W_r.astype(np.float16))
    blr = (b_l + b_r)[None, :].astype(np.float16)
    brmb = (b_r - bias)[None, :].astype(np.float32)
    att_grp = np.tile(att.reshape(1, F), (P, GB)).astype(np.float16)
    ones_row = np.ones((1, P), dtype=np.float16)
    iota = np.arange(P, dtype=np.int32)
    ngroups = nchunk // GB
    W = GB * P
    in_maps = []
    for k in range(N_CORES):
        xsT = x16[src_flat[k]].T                          # [P, nchunk*P]
        xdT = x16[dst_flat[k]].T
        # one-hot mask rows: [P, nchunk*P]; mask[p, c*P + d] = slot(p,c)==d
        sl = slot_flat[k].reshape(nchunk, P)              # [c, p]
        maskH = (sl.T[:, :, None] == iota[None, None, :]).astype(
            np.float16).reshape(P, nchunk * P)
        # interleave per group: [xsT_g | xdT_g | mask_g]
        blob = np.empty((P, ngroups, 3, W), dtype=np.float16)
        blob[:, :, 0, :] = xsT.reshape(P, ngroups, W)
        blob[:, :, 1, :] = xdT.reshape(P, ngroups, W)
        blob[:, :, 2, :] = maskH.reshape(P, ngroups, W)
        xlocT = np.ascontiguousarray(x16[k * NPC:(k + 1) * NPC].T)
        in_maps.append({
            "blob": blob.reshape(P, nchunk * P * 3), "xlocT": xlocT,
            "wl": wl, "wr": wr, "blr_row": blr, "brmb_row": brmb,
            "att_grp": att_grp, "ones_row": ones_row,
        })
    return in_maps


def kernel(x, W_l, b_l, W_r, b_r, att, bias, src, dst):
    x = np.asarray(x, dtype=np.float32)
    W_l = np.asarray(W_l, dtype=np.float32)
    W_r = np.asarray(W_r, dtype=np.float32)
    b_l = np.asarray(b_l, dtype=np.float32)
    b_r = np.asarray(b_r, dtype=np.float32)
    att = np.asarray(att, dtype=np.float32)
    bias = np.asarray(bias, dtype=np.float32)

    src_flat, dst_flat, slot_flat, cpt, cbase, nchunk = _host_prep(src, dst)
    hb_lr = bool(np.any(b_l != 0) or np.any(b_r != 0))
    nc = _build_program(nchunk, cpt, cbase, hb_lr)
    in_maps = _make_in_maps(x, W_l, b_l, W_r, b_r, att, bias,
                            src_flat, dst_flat, slot_flat, nchunk)

    global LAST_NC
    LAST_NC = nc
    from concourse import bass2jax
    results = bass2jax.run_bass_via_pjrt(nc, in_maps, n_cores=N_CORES)

    out = np.empty((N_NODES, F), dtype=np.float32)
    for k in range(N_CORES):
        lo = k * NPC
        hi = min(lo + NPC, N_NODES)
        out[lo:hi] = results[k]["out"][:hi - lo]
    return out
